# revision 29
# baseline (speedup 1.0000x reference)
"""Trainium2 Bass kernel: pairwise squared Euclidean distance (feat vs centroids).

dist[n, k] = ||feat[n]||^2 + ||centers[k]||^2 - 2 * feat[n] . centers[k]

Shapes (hardcoded): feat [16384, 1024] f32, centers [2048, 1024] f32,
output dist [16384, 2048] f32.

Strategy: data-parallel over 8 NeuronCores — each core owns 2048 feat rows and
a replicated copy of the centers, computing its [2048, 2048] block of the
distance matrix.

Per core the kernel is a single large GEMM on the TensorEngine:
  - host pre-transposes both operands so the contraction dim (D) sits on the
    partition axis and quantizes them to fp8e4m3 (feat pre-scaled by -2,
    centers by +256 — both exact powers of two; the PE then accumulates
    -512*cross in f32 PSUM). fp8 with perf_mode=DoubleRow packs two
    contraction rows per PE cell for ~1.5x bf16 matmul throughput,
  - the row norms ||f||^2 (f32, computed on host in f64) are folded in as the
    per-partition bias of the ScalarEngine Identity-activation that evicts
    PSUM (which also applies the 1/256 rescale),
  - the center norms ||c||^2 (f32, broadcast across partitions) are added by a
    VectorEngine tensor_add before each chunk streams back to HBM.

The fp8 quantization error lands entirely on the cross term; dist is dominated
by ||f||^2 ~ 1024, giving ~3e-4 max relative error on the output.
"""

import sys
import types

import numpy as np
import ml_dtypes
from contextlib import ExitStack


def _ensure_axon_hooks_stub():
    # concourse.bass_utils imports antenv.axon_hooks when tracing is requested
    # (BASS_TRACE=1); that module is absent from this image. Provide a stub so
    # a trace request degrades to "no trace" instead of crashing the run.
    try:
        import antenv.axon_hooks  # noqa: F401
    except ImportError:
        m = types.ModuleType("antenv.axon_hooks")
        m._hook = None
        m.set_axon_ntff_profile_hook = lambda h: setattr(m, "_hook", h)
        m.get_axon_ntff_profile_hook = lambda: m._hook
        sys.modules["antenv.axon_hooks"] = m


_ensure_axon_hooks_stub()

import concourse.bass as bass
import concourse.bacc as bacc
import concourse.tile as tile
from concourse import mybir
from concourse.bass_utils import run_bass_kernel_spmd

FP8 = mybir.dt.np(mybir.dt.float8e4)  # ml_dtypes.float8_e4m3

N, K, D = 16384, 2048, 1024
P = 128
NCORES = 8
N_SH = N // NCORES      # 2048 feat rows per core
NT = N_SH // P          # 16 row tiles
DJ = D // P             # 8 contraction tiles
DR = DJ // 2            # 4 DoubleRow accumulation steps
CHUNK = 512             # matmul free dim (one PSUM bank of f32)
CH = K // CHUNK         # 4 k-chunks
CSCALE = 256.0          # centers pre-scale before fp8 quantization (2^8)
OUT_BIAS = 1024.0       # subtracted on device / re-added on host (f16 range)

# Results of the last device run (BassKernelResults); lets a test harness
# opt into tracing via BASS_TRACE=1 and read exec_time_ns afterwards.
LAST_RESULTS = None

_NC_CACHE = None


def _build_nc():
    nc = bacc.Bacc(None, target_bir_lowering=False, debug=False)

    # featT[p, i, dj, n] = -2 * feat[i*128 + n, dj*128 + p]  (fp8) — one row
    # tile i is 1 KB contiguous per partition (big DMA packets).
    featT = nc.declare_dram_parameter("featT", [P, NT, DJ, P], mybir.dt.float8e4, isOutput=False)
    # centsT[p, dj, k] = 256 * centers[k, dj*128 + p]  (fp8)
    centsT = nc.declare_dram_parameter("centsT", [P, DJ, K], mybir.dt.float8e4, isOutput=False)
    # aux[:, :K] = ||c||^2 replicated across partitions; aux[:, K + i] = f2 of
    # row tile i (f2[i*128 + p] at partition p)
    aux = nc.declare_dram_parameter("aux", [P, K + NT], mybir.dt.float16, isOutput=False)
    # Output leaves the device as f16 (dist ~ 1024 -> 2.4e-4 quantization);
    # the host widens to f32. Halves the dominant store traffic.
    dist = nc.declare_dram_parameter("dist", [N_SH, K], mybir.dt.float16, isOutput=True)

    with ExitStack() as ctx:
        tc = ctx.enter_context(tile.TileContext(nc))
        const_pool = ctx.enter_context(tc.tile_pool(name="const_pool", bufs=1))
        ft_pool = ctx.enter_context(tc.tile_pool(name="ft_pool", bufs=4))
        out_pool = ctx.enter_context(tc.tile_pool(name="out_pool", bufs=3))
        psum_pool = ctx.enter_context(tc.tile_pool(name="psum_pool", bufs=8, space="PSUM"))

        # Centers stay resident in SBUF (2 MB fp8), loaded as two 1 MB halves
        # (8 KB contiguous per partition) split across the two HWDGE queues.
        ct_sb = const_pool.tile([P, DJ, K], mybir.dt.float8e4)
        ft_tiles = []
        ft0 = ft_pool.tile([P, 2, DJ, P], mybir.dt.float8e4, name="ftp0", tag="ft")
        # Critical-path order: the first ct half (1 MB) leads the Sync queue,
        # the second half leads the ScalarE queue, and feat tiles stream on
        # the GpSimd SWDGE queue — its instruction stream carries only feat
        # loads, so a tile-pool WAR wait there never head-of-line-blocks the
        # epilogue ops (which live on ScalarE/VectorE).
        nc.gpsimd.dma_start(ft0[:, 0], featT[:, 0, :, :])
        nc.sync.dma_start(ct_sb[:, 0:4, :], centsT[:, 0:4, :])
        nc.scalar.dma_start(ct_sb[:, 4:8, :], centsT[:, 4:8, :])
        nc.gpsimd.dma_start(ft0[:, 1], featT[:, 1, :, :])
        ft_tiles.append(ft0)
        aux_sb = const_pool.tile([P, K + NT], mybir.dt.float16)
        nc.sync.dma_start(aux_sb[:], aux[:, :])
        c2_sb = aux_sb[:, :K]
        f2_sb = aux_sb[:, K : K + NT]

        # PE warmup: junk matmuls that keep the TensorEngine busy while the
        # first real operands stream in, so the HAM clock gate reaches 8/8
        # (2.4 GHz) before the real matmuls start. Results are discarded.
        warm = const_pool.tile([P, 2 * P], mybir.dt.float8e4)
        nc.vector.memset(warm[:], 0.25)
        ps_warm = psum_pool.tile([P, CHUNK], mybir.dt.float32, name="ps_warm", tag="ps")
        for _ in range(142):
            nc.tensor.matmul(
                ps_warm[:, :P], warm[:, :P], warm[:, P:], start=True, stop=True
            )

        for i in range(NT):
            ip, ii = i // 2, i % 2
            if ii == 0 and ip > 0:
                ft = ft_pool.tile([P, 2, DJ, P], mybir.dt.float8e4, name=f"ftp{ip}", tag="ft")
                nc.gpsimd.dma_start(ft[:], featT[:, 2 * ip : 2 * ip + 2, :, :])
                ft_tiles.append(ft)
            ft = ft_tiles[ip]
            out_sb = out_pool.tile([P, K], mybir.dt.float16)
            for c in range(CH):
                ps = psum_pool.tile([P, CHUNK], mybir.dt.float32)
                for t in range(DR):
                    nc.tensor.matmul(
                        ps[:],
                        ft[:, ii, 2 * t : 2 * t + 2, :],
                        ct_sb[:, 2 * t : 2 * t + 2, bass.ts(c, CHUNK)],
                        start=(t == 0),
                        stop=(t == DR - 1),
                        perf_mode=mybir.MatmulPerfMode.DoubleRow,
                    )
                # psum holds -2*CSCALE*cross; evict with the 1/CSCALE rescale
                # plus the (f2 - OUT_BIAS) per-partition bias (ScalarE), then
                # add c2 (free-dim vector, VectorE). The row tile streams out
                # as one f16 store once all four chunks are done.
                chunk = out_sb[:, bass.ts(c, CHUNK)]
                nc.scalar.activation(
                    chunk, ps[:], mybir.ActivationFunctionType.Identity,
                    bias=f2_sb[:, i : i + 1], scale=1.0 / CSCALE,
                )
                nc.vector.tensor_add(chunk, chunk, c2_sb[:, bass.ts(c, CHUNK)])
            nc.sync.dma_start(dist[bass.ts(i, P), :], out_sb[:])
    nc.compile()
    return nc


def kernel(feat, centers):
    global LAST_RESULTS, _NC_CACHE
    feat = np.ascontiguousarray(np.asarray(feat, dtype=np.float32))
    centers = np.ascontiguousarray(np.asarray(centers, dtype=np.float32))
    assert feat.shape == (N, D) and centers.shape == (K, D)

    f2 = np.einsum("nd,nd->n", feat, feat, dtype=np.float64).astype(np.float32)
    c2 = np.einsum("kd,kd->k", centers, centers, dtype=np.float64).astype(np.float32)

    # [K, D] -> [D, K] -> [P, DJ, K] with partition idx innermost in D
    ctT = np.ascontiguousarray(
        (CSCALE * centers.T).astype(FP8).reshape(DJ, P, K).transpose(1, 0, 2)
    )
    featm2 = (-2.0 * feat).astype(FP8)

    in_maps = []
    for s in range(NCORES):
        rows = slice(s * N_SH, (s + 1) * N_SH)
        # [N_SH, D] -> [p, i, dj, n]: featT[p, i, dj, n] = featm2[i*128+n, dj*128+p]
        ftT = np.ascontiguousarray(
            featm2[rows].reshape(NT, P, DJ, P).transpose(3, 0, 2, 1)
        )
        # f16 aux: c2 ~ 0.33 and the f2 residual is within +-250, so f16
        # quantization contributes <1e-4 relative error on dist.
        auxm = np.empty((P, K + NT), np.float16)
        auxm[:, :K] = c2[None, :]
        # OUT_BIAS is subtracted on-device (folded into f2) and added back on
        # host, so the f16 output carries only the small residual — shrinking
        # its quantization step ~4x.
        auxm[:, K:] = (f2[rows].reshape(NT, P).T - OUT_BIAS).astype(np.float16)
        in_maps.append({"featT": ftT, "centsT": ctT, "aux": auxm})

    if _NC_CACHE is None:
        _NC_CACHE = _build_nc()
    res = run_bass_kernel_spmd(_NC_CACHE, in_maps, core_ids=list(range(NCORES)))
    LAST_RESULTS = res
    out = np.concatenate([res.results[s]["dist"] for s in range(NCORES)], axis=0)
    return out.astype(np.float32) + np.float32(OUT_BIAS)


# revision 31
# speedup vs baseline: 1.0470x; 1.0470x over previous
"""Trainium2 Bass kernel: pairwise squared Euclidean distance (feat vs centroids).

dist[n, k] = ||feat[n]||^2 + ||centers[k]||^2 - 2 * feat[n] . centers[k]

Shapes (hardcoded): feat [16384, 1024] f32, centers [2048, 1024] f32,
output dist [16384, 2048] f32.

Strategy: data-parallel over 8 NeuronCores — each core owns 2048 feat rows and
a replicated copy of the centers, computing its [2048, 2048] block of the
distance matrix.

Per core the kernel is a single large GEMM on the TensorEngine:
  - host pre-transposes both operands so the contraction dim (D) sits on the
    partition axis and quantizes them to fp8e4m3 (feat pre-scaled by -2,
    centers by +256 — both exact powers of two; the PE then accumulates
    -512*cross in f32 PSUM). fp8 with perf_mode=DoubleRow packs two
    contraction rows per PE cell for ~1.5x bf16 matmul throughput,
  - the row norms ||f||^2 (f32, computed on host in f64) are folded in as the
    per-partition bias of the ScalarEngine Identity-activation that evicts
    PSUM (which also applies the 1/256 rescale),
  - the center norms ||c||^2 (f32, broadcast across partitions) are added by a
    VectorEngine tensor_add before each chunk streams back to HBM.

The fp8 quantization error lands entirely on the cross term; dist is dominated
by ||f||^2 ~ 1024, giving ~3e-4 max relative error on the output.
"""

import sys
import types

import numpy as np
import ml_dtypes
from contextlib import ExitStack


def _ensure_axon_hooks_stub():
    # concourse.bass_utils imports antenv.axon_hooks when tracing is requested
    # (BASS_TRACE=1); that module is absent from this image. Provide a stub so
    # a trace request degrades to "no trace" instead of crashing the run.
    try:
        import antenv.axon_hooks  # noqa: F401
    except ImportError:
        m = types.ModuleType("antenv.axon_hooks")
        m._hook = None
        m.set_axon_ntff_profile_hook = lambda h: setattr(m, "_hook", h)
        m.get_axon_ntff_profile_hook = lambda: m._hook
        sys.modules["antenv.axon_hooks"] = m


_ensure_axon_hooks_stub()

import concourse.bass as bass
import concourse.bacc as bacc
import concourse.tile as tile
from concourse import mybir
from concourse.bass_utils import run_bass_kernel_spmd

FP8 = mybir.dt.np(mybir.dt.float8e4)  # ml_dtypes.float8_e4m3

N, K, D = 16384, 2048, 1024
P = 128
NCORES = 8
N_SH = N // NCORES      # 2048 feat rows per core
NT = N_SH // P          # 16 row tiles
DJ = D // P             # 8 contraction tiles
DR = DJ // 2            # 4 DoubleRow accumulation steps
CHUNK = 512             # matmul free dim (one PSUM bank of f32)
CH = K // CHUNK         # 4 k-chunks
CSCALE = 256.0          # centers pre-scale before fp8 quantization (2^8)
OUT_BIAS = 1024.0       # subtracted on device / re-added on host (f16 range)

# Results of the last device run (BassKernelResults); lets a test harness
# opt into tracing via BASS_TRACE=1 and read exec_time_ns afterwards.
LAST_RESULTS = None

_NC_CACHE = None


def _build_nc():
    nc = bacc.Bacc(None, target_bir_lowering=False, debug=False)

    # featT[p, i, dj, n] = -2 * feat[i*128 + n, dj*128 + p]  (fp8) — one row
    # tile i is 1 KB contiguous per partition (big DMA packets).
    featT = nc.declare_dram_parameter("featT", [P, NT, DJ, P], mybir.dt.float8e4, isOutput=False)
    # centsT[p, dj, k] = 256 * centers[k, dj*128 + p]  (fp8)
    centsT = nc.declare_dram_parameter("centsT", [P, DJ, K], mybir.dt.float8e4, isOutput=False)
    # aux[:, :K] = ||c||^2 replicated across partitions; aux[:, K + i] = f2 of
    # row tile i (f2[i*128 + p] at partition p)
    aux = nc.declare_dram_parameter("aux", [P, K + NT], mybir.dt.float16, isOutput=False)
    # Output leaves the device as f16 (dist ~ 1024 -> 2.4e-4 quantization);
    # the host widens to f32. Halves the dominant store traffic.
    dist = nc.declare_dram_parameter("dist", [N_SH, K], mybir.dt.float16, isOutput=True)

    with ExitStack() as ctx:
        tc = ctx.enter_context(tile.TileContext(nc))
        const_pool = ctx.enter_context(tc.tile_pool(name="const_pool", bufs=1))
        ft_pool = ctx.enter_context(tc.tile_pool(name="ft_pool", bufs=4))
        out_pool = ctx.enter_context(tc.tile_pool(name="out_pool", bufs=3))
        psum_pool = ctx.enter_context(tc.tile_pool(name="psum_pool", bufs=8, space="PSUM"))

        # Centers stay resident in SBUF (2 MB fp8), loaded as two 1 MB halves
        # (8 KB contiguous per partition) split across the two HWDGE queues.
        ct_sb = const_pool.tile([P, DJ, K], mybir.dt.float8e4)
        ft_tiles = []
        ft0 = ft_pool.tile([P, 2, DJ, P], mybir.dt.float8e4, name="ftp0", tag="ft")
        # Critical-path order: the first ct half (1 MB) leads the Sync queue,
        # the second half leads the ScalarE queue, and feat tiles stream on
        # the GpSimd SWDGE queue — its instruction stream carries only feat
        # loads, so a tile-pool WAR wait there never head-of-line-blocks the
        # epilogue ops (which live on ScalarE/VectorE).
        nc.gpsimd.dma_start(ft0[:, 0], featT[:, 0, :, :])
        nc.sync.dma_start(ct_sb[:, 0:4, :], centsT[:, 0:4, :])
        nc.scalar.dma_start(ct_sb[:, 4:8, :], centsT[:, 4:8, :])
        nc.gpsimd.dma_start(ft0[:, 1], featT[:, 1, :, :])
        ft_tiles.append(ft0)
        aux_sb = const_pool.tile([P, K + NT], mybir.dt.float16)
        nc.sync.dma_start(aux_sb[:], aux[:, :])
        c2_sb = aux_sb[:, :K]
        f2_sb = aux_sb[:, K : K + NT]

        # PE warmup: junk matmuls that keep the TensorEngine busy while the
        # first real operands stream in, so the HAM clock gate reaches 8/8
        # (2.4 GHz) before the real matmuls start. Results are discarded.
        warm = const_pool.tile([P, 2 * P], mybir.dt.float8e4)
        nc.vector.memset(warm[:], 0.25)
        ps_warm = psum_pool.tile([P, CHUNK], mybir.dt.float32, name="ps_warm", tag="ps")
        for _ in range(130):
            nc.tensor.matmul(
                ps_warm[:, :P], warm[:, :P], warm[:, P:], start=True, stop=True
            )

        for i in range(NT):
            ip, ii = i // 2, i % 2
            if ii == 0 and ip > 0:
                ft = ft_pool.tile([P, 2, DJ, P], mybir.dt.float8e4, name=f"ftp{ip}", tag="ft")
                nc.gpsimd.dma_start(ft[:], featT[:, 2 * ip : 2 * ip + 2, :, :])
                ft_tiles.append(ft)
            ft = ft_tiles[ip]
            out_sb = out_pool.tile([P, K], mybir.dt.float16)
            for c in range(CH):
                ps = psum_pool.tile([P, CHUNK], mybir.dt.float32)
                for t in range(DR):
                    nc.tensor.matmul(
                        ps[:],
                        ft[:, ii, 2 * t : 2 * t + 2, :],
                        ct_sb[:, 2 * t : 2 * t + 2, bass.ts(c, CHUNK)],
                        start=(t == 0),
                        stop=(t == DR - 1),
                        perf_mode=mybir.MatmulPerfMode.DoubleRow,
                    )
                # psum holds -2*CSCALE*cross; evict with the 1/CSCALE rescale
                # plus the (f2 - OUT_BIAS) per-partition bias (ScalarE), then
                # add c2 (free-dim vector, VectorE). The row tile streams out
                # as one f16 store once all four chunks are done.
                chunk = out_sb[:, bass.ts(c, CHUNK)]
                nc.scalar.activation(
                    chunk, ps[:], mybir.ActivationFunctionType.Identity,
                    bias=f2_sb[:, i : i + 1], scale=1.0 / CSCALE,
                )
                nc.vector.tensor_add(chunk, chunk, c2_sb[:, bass.ts(c, CHUNK)])
                if i == NT - 1:
                    # Last row tile: store per chunk so the final drain starts
                    # as soon as each chunk's epilogue lands, not after all 4.
                    nc.sync.dma_start(
                        dist[bass.ts(i, P), bass.ts(c, CHUNK)], chunk
                    )
            if i < NT - 1:
                nc.sync.dma_start(dist[bass.ts(i, P), :], out_sb[:])
    nc.compile()
    return nc


def kernel(feat, centers):
    global LAST_RESULTS, _NC_CACHE
    feat = np.ascontiguousarray(np.asarray(feat, dtype=np.float32))
    centers = np.ascontiguousarray(np.asarray(centers, dtype=np.float32))
    assert feat.shape == (N, D) and centers.shape == (K, D)

    f2 = np.einsum("nd,nd->n", feat, feat, dtype=np.float64).astype(np.float32)
    c2 = np.einsum("kd,kd->k", centers, centers, dtype=np.float64).astype(np.float32)

    # [K, D] -> [D, K] -> [P, DJ, K] with partition idx innermost in D
    ctT = np.ascontiguousarray(
        (CSCALE * centers.T).astype(FP8).reshape(DJ, P, K).transpose(1, 0, 2)
    )
    featm2 = (-2.0 * feat).astype(FP8)

    in_maps = []
    for s in range(NCORES):
        rows = slice(s * N_SH, (s + 1) * N_SH)
        # [N_SH, D] -> [p, i, dj, n]: featT[p, i, dj, n] = featm2[i*128+n, dj*128+p]
        ftT = np.ascontiguousarray(
            featm2[rows].reshape(NT, P, DJ, P).transpose(3, 0, 2, 1)
        )
        # f16 aux: c2 ~ 0.33 and the f2 residual is within +-250, so f16
        # quantization contributes <1e-4 relative error on dist.
        auxm = np.empty((P, K + NT), np.float16)
        auxm[:, :K] = c2[None, :]
        # OUT_BIAS is subtracted on-device (folded into f2) and added back on
        # host, so the f16 output carries only the small residual — shrinking
        # its quantization step ~4x.
        auxm[:, K:] = (f2[rows].reshape(NT, P).T - OUT_BIAS).astype(np.float16)
        in_maps.append({"featT": ftT, "centsT": ctT, "aux": auxm})

    if _NC_CACHE is None:
        _NC_CACHE = _build_nc()
    res = run_bass_kernel_spmd(_NC_CACHE, in_maps, core_ids=list(range(NCORES)))
    LAST_RESULTS = res
    out = np.concatenate([res.results[s]["dist"] for s in range(NCORES)], axis=0)
    return out.astype(np.float32) + np.float32(OUT_BIAS)
